# revision 1
# baseline (speedup 1.0000x reference)
"""Bass/Trainium2 kernel for the attention-decoder problem.

Data-parallel over batch: 8 cores x 32 batch each. Each core runs the full
26-step Bahdanau-attention + GRU scan on its batch shard and the generator
GEMM, producing its contiguous slice of the output rows (row = b*L + i,
b-major, so shards concatenate).
"""

import numpy as np
import ml_dtypes

import concourse.bass as bass
import concourse.mybir as mybir
import concourse.tile as tile
from concourse import bacc
from concourse.bass_utils import run_bass_kernel_spmd

BF16 = mybir.dt.bfloat16
F32 = mybir.dt.float32
AF = mybir.ActivationFunctionType
ALU = mybir.AluOpType

T, BFULL, C = 128, 256, 512
H, L, NCLS = 512, 26, 6736
NCORES = 8
B = BFULL // NCORES          # 32 per core
ROWS = B * L                 # 832 output rows per core
HC = H // 128                # 4 h-chunks
CC = C // 128                # 4 c-chunks
G3 = 3 * H                   # 1536
RB = 4                       # batches per gen row-chunk
RCH = ROWS // (RB * L) if False else B // RB   # 8 row-chunks of RB*L=104 rows
GEN_M = RB * L               # 104
NSUB = 512                   # max matmul free dim (one PSUM bank of f32)
TRACE = False                # set by test.py for profiling runs


def _bcast_part(ap, count):
    """Broadcast a [1, f...] AP across `count` partitions (stride-0 partition)."""
    return ap.partition_broadcast(count)


def build_graph(reps=1):
    nc = bacc.Bacc(None, target_bir_lowering=False, debug=False)

    # ---- DRAM parameters (per-core shard shapes) ----
    featsT = nc.declare_dram_parameter("featsT", [C, T, B], BF16, isOutput=False)   # [c, t, b]
    featsS = nc.declare_dram_parameter("featsS", [B, T, C], BF16, isOutput=False)   # [b, t, c]
    wi2h = nc.declare_dram_parameter("wi2h", [C, H], BF16, isOutput=False)          # k=c, m=h
    wh2h = nc.declare_dram_parameter("wh2h", [H, H], BF16, isOutput=False)          # k=h, m=h'
    bh2h = nc.declare_dram_parameter("bh2h", [128, HC], F32, isOutput=False)
    wrep = nc.declare_dram_parameter("wrep", [H, B], BF16, isOutput=False)          # w_score rep'd
    wih = nc.declare_dram_parameter("wih", [C, G3], BF16, isOutput=False)           # k=c, n=g
    whh = nc.declare_dram_parameter("whh", [H, G3], BF16, isOutput=False)
    brow = nc.declare_dram_parameter("brow", [B, 4 * H], F32, isOutput=False)
    wgen = nc.declare_dram_parameter("wgen", [H, NCLS], BF16, isOutput=False)       # k=h, n=cls
    bgen = nc.declare_dram_parameter("bgen", [GEN_M, NCLS], F32, isOutput=False)
    ident = nc.declare_dram_parameter("ident", [128, 128], BF16, isOutput=False)
    out = nc.declare_dram_parameter("out", [ROWS, NCLS], F32, isOutput=True)

    with tile.TileContext(nc) as tc:
      for rep in range(reps):
        with tc.tile_pool(name=f"keep{rep}", bufs=1) as kp:
            hist = kp.tile([128, HC, B, L], BF16)         # h^T history (b, step)
            h0_sb = kp.tile([128, HC, B], BF16)           # zeros

            with (
                tc.tile_pool(name=f"spp{rep}", bufs=1) as pp,
                tc.tile_pool(name=f"zpool{rep}", bufs=5) as zp,
                tc.tile_pool(name=f"psE{rep}", bufs=4, space="PSUM") as psE,
                tc.tile_pool(name=f"psS{rep}", bufs=2, space="PSUM") as psS,
                tc.tile_pool(name=f"psC{rep}", bufs=2, space="PSUM") as psC,
            ):
                # ---- persistent (scan-scope) SBUF ----
                P_sb = pp.tile([128, HC, T, B], BF16)         # proj^T [h | t, b]
                fs_sb = pp.tile([128, B, C], BF16)            # feats  [t | b, c]
                wh2h_sb = pp.tile([128, HC, H], BF16)
                bh2h_sb = pp.tile([128, HC], F32)
                wrep_sb = pp.tile([128, HC, B], BF16)
                wih_sb = pp.tile([128, CC, G3], BF16)
                whh_sb = pp.tile([128, HC, G3], BF16)
                brow_sb = pp.tile([B, 4 * H], F32)    # [brz | bin | bhn] replicated
                ones_sb = pp.tile([B, 128], F32)
                r128_sb = pp.tile([128, B], F32)
                ident_sb = pp.tile([B, B], BF16)
                h_bh = pp.tile([B, H], BF16)                  # h in [b, h]
                E_sb = pp.tile([B, T, B], BF16)               # exp(e) [32 | t, b]
                hpT_sb = pp.tile([128, HC, B], BF16)
                ctxT_sb = pp.tile([128, CC, B], BF16)
                aT_sb = pp.tile([128, 2, B], BF16)            # raw | normalized
                sr_sb = pp.tile([B, B], F32)                  # s replicated
                spart_sb = pp.tile([B, 8, B], F32)            # per-sub partial sums
                rz_sb = pp.tile([B, 2 * H], F32)
                An_sb = pp.tile([B, H], F32)
                Bn_sb = pp.tile([B, H], F32)

                # ---- load weights/constants ----
                for kc in range(CC):
                    nc.sync.dma_start(wih_sb[:, kc, :], wih[kc * 128:(kc + 1) * 128, :])
                for kc in range(HC):
                    nc.sync.dma_start(wh2h_sb[:, kc, :], wh2h[kc * 128:(kc + 1) * 128, :])
                    nc.sync.dma_start(whh_sb[:, kc, :], whh[kc * 128:(kc + 1) * 128, :])
                    nc.sync.dma_start(wrep_sb[:, kc, :], wrep[kc * 128:(kc + 1) * 128, :])
                nc.sync.dma_start(bh2h_sb[:, :], bh2h[:, :])
                nc.sync.dma_start(brow_sb[:, :], brow[:, :])
                nc.vector.memset(ones_sb[:, :], 1.0 / B)
                nc.sync.dma_start(ident_sb[:, :], ident[0:B, 0:B])
                for b in range(B):
                    nc.sync.dma_start(fs_sb[:, b, :], featsS[b, :, :])

                nc.vector.memset(h0_sb[:, :, :], 0.0)
                nc.vector.memset(h_bh[:, :], 0.0)

                # ---- prologue: P = W_i2h @ feats^T, laid out [h | t, b] ----
                with tc.tile_pool(name=f"ft{rep}", bufs=1) as fp:
                    ft_sb = fp.tile([128, CC, T, B], BF16)
                    wi2h_sb = fp.tile([128, CC, H], BF16)
                    for kc in range(CC):
                        nc.sync.dma_start(wi2h_sb[:, kc, :],
                                          wi2h[kc * 128:(kc + 1) * 128, :])
                        nc.sync.dma_start(ft_sb[:, kc, :, :],
                                          featsT[kc * 128:(kc + 1) * 128, :, :])
                    for mc in range(HC):
                        for ns in range(T * B // NSUB):       # 8 subs of 512 = 16 t
                            ppsum = psE.tile([128, NSUB], F32, tag="e")
                            for kc in range(CC):
                                nc.tensor.matmul(
                                    ppsum[:, :],
                                    wi2h_sb[:, kc, mc * 128:(mc + 1) * 128],
                                    ft_sb[:, kc, ns * 16:(ns + 1) * 16, :],
                                    start=(kc == 0), stop=(kc == CC - 1),
                                )
                            nc.vector.tensor_copy(
                                P_sb[:, mc, ns * 16:(ns + 1) * 16, :], ppsum[:, :])

                # ---- scan ----
                for k in range(L):
                    def hprev(kc, _k=k):
                        return (h0_sb[:, kc, :] if _k == 0
                                else hist[:, kc, :, _k - 1])
                    # hp^T = W_h2h @ h^T + b_h2h   -> [h | b]
                    for mc in range(HC):
                        hp_ps = psS.tile([128, B], F32, tag="small")
                        for kc in range(HC):
                            nc.tensor.matmul(
                                hp_ps[:, :],
                                wh2h_sb[:, kc, mc * 128:(mc + 1) * 128],
                                hprev(kc),
                                start=(kc == 0), stop=(kc == HC - 1),
                            )
                        nc.scalar.activation(hpT_sb[:, mc, :], hp_ps[:, :],
                                             AF.Identity, bias=bh2h_sb[:, mc:mc + 1])

                    # z = tanh(P + hp)  per h-chunk; e = w . z  via PE
                    z_tiles = []
                    for hc in range(HC):
                        z = zp.tile([128, T, B], BF16, tag="z")
                        nc.vector.tensor_tensor(
                            z[:, :, :], P_sb[:, hc, :, :],
                            hpT_sb[:, hc:hc + 1, :].broadcast_to((128, T, B)),
                            op=ALU.add)
                        nc.scalar.activation(z[:, :, :], z[:, :, :], AF.Tanh)
                        z_tiles.append(z)

                    for ns in range(T * B // NSUB):           # 8 subs (16 t each)
                        e_ps = psE.tile([B, NSUB], F32, tag="e")
                        for hc in range(HC):
                            nc.tensor.matmul(
                                e_ps[:, :],
                                wrep_sb[:, hc, :],
                                z_tiles[hc][:, ns * 16:(ns + 1) * 16, :],
                                start=(hc == 0), stop=(hc == HC - 1),
                            )
                        nc.scalar.activation(
                            E_sb[:, ns * 16:(ns + 1) * 16, :], e_ps[:, :], AF.Exp)

                    # s[b] = sum_t E (rows replicated); r = 1/s
                    for ns in range(8):
                        part = E_sb[:, ns * 16:(ns + 1) * 16, :]
                        nc.vector.tensor_reduce(
                            spart_sb[:, ns, :], part.rearrange("p t b -> p b t"),
                            axis=mybir.AxisListType.X, op=ALU.add)
                    nc.vector.tensor_reduce(
                        sr_sb[:, :], spart_sb[:, :, :].rearrange("p n b -> p b n"),
                        axis=mybir.AxisListType.X, op=ALU.add)
                    # replicate s across 128 partitions: (1/B)*ones^T @ s_rep
                    s128_ps = psS.tile([128, B], F32, tag="small")
                    nc.tensor.matmul(s128_ps[:, :], ones_sb[:, :], sr_sb[:, :],
                                     start=True, stop=True)
                    nc.vector.reciprocal(r128_sb[:, :], s128_ps[:, :])

                    # alpha^T via SBUF->SBUF DMA off row 0 of E, then scale by r
                    nc.sync.dma_start(aT_sb[:, 0, :], E_sb[0:1, :, :])
                    nc.vector.tensor_tensor(
                        aT_sb[:, 1, :], aT_sb[:, 0, :], r128_sb[:, :], op=ALU.mult)

                    # context^T[c, b] = feats_b^T @ alpha_b  (per-b matmuls)
                    ctx_ps = psC.tile([128, CC, B], F32, tag="ctx")
                    for b in range(B):
                        for cc in range(CC):
                            nc.tensor.matmul(
                                ctx_ps[:, cc, b:b + 1],
                                fs_sb[:, b, cc * 128:(cc + 1) * 128],
                                aT_sb[:, 1, b:b + 1],
                                start=True, stop=True,
                            )
                    nc.vector.tensor_copy(ctxT_sb[:, :, :], ctx_ps[:, :, :])

                    # gates: rz = sigmoid(Wih_rz ctx + Whh_rz h + b)
                    for half in range(2):
                        rz_ps = psE.tile([B, NSUB], F32, tag="e")
                        for kc in range(CC):
                            nc.tensor.matmul(
                                rz_ps[:, :], ctxT_sb[:, kc, :],
                                wih_sb[:, kc, half * NSUB:(half + 1) * NSUB],
                                start=(kc == 0), stop=False)
                        for kc in range(HC):
                            nc.tensor.matmul(
                                rz_ps[:, :], hprev(kc),
                                whh_sb[:, kc, half * NSUB:(half + 1) * NSUB],
                                start=False, stop=(kc == HC - 1))
                        nc.vector.tensor_tensor(
                            rz_sb[:, half * NSUB:(half + 1) * NSUB], rz_ps[:, :],
                            brow_sb[:, half * NSUB:(half + 1) * NSUB],
                            op=ALU.add)
                    nc.scalar.activation(rz_sb[:, :], rz_sb[:, :], AF.Sigmoid)

                    gn_ps = psE.tile([B, H], F32, tag="e")
                    for kc in range(CC):
                        nc.tensor.matmul(gn_ps[:, :], ctxT_sb[:, kc, :],
                                         wih_sb[:, kc, 2 * H:], start=(kc == 0),
                                         stop=(kc == CC - 1))
                    hn_ps = psE.tile([B, H], F32, tag="e")
                    for kc in range(HC):
                        nc.tensor.matmul(hn_ps[:, :], hprev(kc),
                                         whh_sb[:, kc, 2 * H:], start=(kc == 0),
                                         stop=(kc == HC - 1))
                    nc.vector.tensor_tensor(
                        An_sb[:, :], gn_ps[:, :],
                        brow_sb[:, 2 * H:3 * H], op=ALU.add)
                    nc.vector.tensor_tensor(
                        Bn_sb[:, :], hn_ps[:, :],
                        brow_sb[:, 3 * H:4 * H], op=ALU.add)
                    nc.vector.tensor_tensor(Bn_sb[:, :], rz_sb[:, 0:H], Bn_sb[:, :],
                                            op=ALU.mult)
                    nc.vector.tensor_tensor(An_sb[:, :], An_sb[:, :], Bn_sb[:, :],
                                            op=ALU.add)
                    nc.scalar.activation(An_sb[:, :], An_sb[:, :], AF.Tanh)
                    # h' = n + z * (h - n);  An holds n
                    nc.vector.tensor_tensor(Bn_sb[:, :], h_bh[:, :], An_sb[:, :],
                                            op=ALU.subtract)
                    nc.vector.tensor_tensor(Bn_sb[:, :], rz_sb[:, H:], Bn_sb[:, :],
                                            op=ALU.mult)
                    nc.vector.tensor_tensor(h_bh[:, :], An_sb[:, :], Bn_sb[:, :],
                                            op=ALU.add)

                    # h^T for next step + history (PE transpose per h-chunk)
                    for hc in range(HC):
                        ht_ps = psS.tile([128, B], BF16, tag="small")
                        nc.tensor.transpose(
                            ht_ps[:, :], h_bh[:, hc * 128:(hc + 1) * 128],
                            ident_sb[:, :])
                        nc.vector.tensor_copy(hist[:, hc, :, k], ht_ps[:, :])

            # ---- generator ----
            with (
                tc.tile_pool(name=f"genw{rep}", bufs=1) as gwp,
                tc.tile_pool(name=f"geno{rep}", bufs=4) as gop,
                tc.tile_pool(name=f"genp{rep}", bufs=4, space="PSUM") as gpp,
            ):
                wgen_sb = gwp.tile([128, HC, NCLS], BF16)
                bgen_sb = gwp.tile([GEN_M, NCLS], F32)
                for kc in range(HC):
                    nc.sync.dma_start(wgen_sb[:, kc, :],
                                      wgen[kc * 128:(kc + 1) * 128, :])
                nc.sync.dma_start(bgen_sb[:, :], bgen[:, :])

                nsubs = [(i * NSUB, min(NSUB, NCLS - i * NSUB))
                         for i in range((NCLS + NSUB - 1) // NSUB)]
                for rc in range(B // RB):
                    for (off, width) in nsubs:
                        o_ps = gpp.tile([GEN_M, NSUB], F32, tag="op")
                        for kc in range(HC):
                            nc.tensor.matmul(
                                o_ps[:, 0:width],
                                hist[:, kc, rc * RB:(rc + 1) * RB, :],
                                wgen_sb[:, kc, off:off + width],
                                start=(kc == 0), stop=(kc == HC - 1))
                        o_sb = gop.tile([GEN_M, NSUB], F32, tag="ob")
                        nc.vector.tensor_tensor(
                            o_sb[:, 0:width], o_ps[:, 0:width],
                            bgen_sb[:, off:off + width],
                            op=ALU.add)
                        nc.sync.dma_start(
                            out[rc * GEN_M:(rc + 1) * GEN_M, off:off + width],
                            o_sb[:, 0:width])

    nc.finalize()
    return nc


def _get_graph():
    if not hasattr(_get_graph, "_nc"):
        _get_graph._nc = build_graph()
    return _get_graph._nc


def make_in_maps(feats, text_length, W_i2h, W_h2h, b_h2h, W_score, W_ih, W_hh,
                 b_ih, b_hh, W_gen, b_gen):
    bf = ml_dtypes.bfloat16
    feats = np.asarray(feats, np.float32)

    wi2h = np.ascontiguousarray(np.asarray(W_i2h, np.float32).T).astype(bf)
    wh2h = np.ascontiguousarray(np.asarray(W_h2h, np.float32).T).astype(bf)
    bh2h = np.ascontiguousarray(
        np.asarray(b_h2h, np.float32).reshape(HC, 128).T)
    w = np.asarray(W_score, np.float32)[0]
    wrep = np.repeat(w[:, None], B, axis=1).astype(bf)
    wih = np.ascontiguousarray(np.asarray(W_ih, np.float32).T).astype(bf)
    whh = np.ascontiguousarray(np.asarray(W_hh, np.float32).T).astype(bf)
    b_ih = np.asarray(b_ih, np.float32)
    b_hh = np.asarray(b_hh, np.float32)
    brow1 = np.concatenate([b_ih[:2 * H] + b_hh[:2 * H],
                            b_ih[2 * H:], b_hh[2 * H:]])
    brow = np.repeat(brow1[None, :], B, axis=0).astype(np.float32)
    wgen = np.ascontiguousarray(np.asarray(W_gen, np.float32).T).astype(bf)
    bgen = np.repeat(np.asarray(b_gen, np.float32)[None, :], GEN_M, axis=0)
    ident = np.eye(128, dtype=np.float32).astype(bf)

    in_maps = []
    for c in range(NCORES):
        fsh = feats[:, c * B:(c + 1) * B, :]                     # [T, B, C]
        featsT = np.ascontiguousarray(fsh.transpose(2, 0, 1)).astype(bf)
        featsS = np.ascontiguousarray(fsh.transpose(1, 0, 2)).astype(bf)
        in_maps.append({
            "featsT": featsT, "featsS": featsS, "wi2h": wi2h, "wh2h": wh2h,
            "bh2h": bh2h, "wrep": wrep, "wih": wih, "whh": whh, "brow": brow,
            "wgen": wgen, "bgen": bgen, "ident": ident,
        })

    return in_maps


def kernel(**inputs):
    nc = _get_graph()
    in_maps = make_in_maps(**inputs)
    res = run_bass_kernel_spmd(nc, in_maps, core_ids=list(range(NCORES)))
    return np.concatenate([res.results[c]["out"] for c in range(NCORES)], axis=0)

